# revision 50
# baseline (speedup 1.0000x reference)
"""BiRNN (bidirectional GRU) language model kernel for Trainium2, 8 NeuronCores.

Sharding: data-parallel over batch (2 of 16 batch columns per core), no
collectives.  Each core computes the embedding gather, both GRU scans, the
vocab projection and the log-softmax for its 512 tokens.

Key structure:
  - Chunked scan: each direction's 256-step recurrence is split into C chunks
    run in parallel as lanes of the same instructions, each with a W-step
    warmup ramp.  Positions before the sequence start are padded with a
    frozen-gate column (z-gate pre-activation +30 => z=1 => h stays exactly 0),
    so chunks whose warmup crosses position 0 are exact and later chunks
    converge through the GRU's state contraction (validated: W=32 gives
    ~5e-7 max h error).  Serial steps: 256 -> 256/C + W.
  - Per step, the gate matmul Whh @ h(t-1) is split into an accumulation of
    three PSUM matmuls: identity @ gx(t) (the precomputed input gates),
    0.5*Whh @ zh'(t-1) and Whh @ cn(t-1), where h = cn + 0.5*zh'
    (cn = (1-z)*n, zh' = 2z*h).  This keeps the recurrence's serial path at
    mm -> tanh(rz) -> (r+1)*hn -> +xn -> tanh -> *(1-z) -> mm.
  - h states land in a position-indexed buffer (later chunks' warmup writes
    are overwritten by the owning chunk's real writes, which come later in
    step order).
  - Vocab projection: h split bf16 hi/lo packed into one K=128 stationary
    tile per 128 tokens ([h-hi 64 | ones | h-lo 63]); wout packed to match
    ([w-hi 64 | bias | w-hi 63]) so one bf16 matmul per 512 columns gives
    logits to ~2e-3 abs.  wout ([128, V] bf16, 12.9MB) is fully cached in
    SBUF, loaded during the scan.
  - log-softmax without a max pass (|logits| <= 65 so exp can't overflow):
    pass 1 Exp+accum_out per 2048-col group; pass 2 recomputes logits and
    writes logits - log(sum) as bf16 (host converts to f32).  Pass 2 of
    shell k is interleaved with pass 1 of shell k+1; the finalize add is
    split across DVE and Pool so ACT only does the Exps.
"""

import os
import sys
from contextlib import ExitStack

import numpy as np

for _p in (
    "/opt/trn_rl_repo",
    "/root/.axon_site",
    "/root/.axon_site/_ro/trn_rl_repo",
    "/root/.axon_site/_ro/pypackages",
):
    if os.path.isdir(_p) and _p not in sys.path:
        sys.path.append(_p)

import concourse.bass as bass
import concourse.bacc as bacc
import concourse.tile as tile
from concourse import mybir
from concourse.masks import make_identity

F32 = mybir.dt.float32
BF16 = mybir.dt.bfloat16
I32 = mybir.dt.int32
AF = mybir.ActivationFunctionType
ALU = mybir.AluOpType

V = 50257
E = 64
H = 32
S = 256
B = 16
NCORES = 8
BC = B // NCORES          # batch columns per core
T = S * BC                # tokens per core
G3 = 3 * H                # 96 gate rows (r, z, n)

C = int(os.environ.get("KC", "32"))    # chunks per direction
W = int(os.environ.get("KW", "12"))    # warmup steps
P = S // C                              # positions per chunk
STEPS = P + W
LPD = 2 * C                             # lanes per direction (chunk x batchcol)
LAN = 2 * LPD                           # total lanes (L block, R block)
GXW = 2 * W + 2 * S                     # compact gx width per direction block
HBW = 2 * S + 4 * W                     # h buffer width (L pad left, R pad right)

VGRP = int(os.environ.get("KVGRP", "1024"))  # vocab columns per projection group
NPSB = int(os.environ.get("KNPSB", "4"))     # projection PSUM buffers
NGRP = (V + VGRP - 1) // VGRP


def ap3(base, offset, dims):
    """AP with the partition entry of ``base`` plus custom free dims."""
    return bass.AP(tensor=base.tensor, offset=base.offset + offset,
                   ap=[list(base.ap[0])] + [list(d) for d in dims])


def build_module(phases=("pre", "scan", "proj"),
                 fin_split=tuple(os.environ.get("KFIN", "v"))):
    nc = bacc.Bacc("TRN2", target_bir_lowering=False)
    tok_h = nc.dram_tensor("tok", (T,), I32, kind="ExternalInput")
    emb_h = nc.dram_tensor("embed", (V, E), F32, kind="ExternalInput")
    wih_h = nc.dram_tensor("wih", (E + 1, 2 * G3), F32, kind="ExternalInput")
    whhdd_h = nc.dram_tensor("whhdd", (H + 1, 2 * G3), F32, kind="ExternalInput")
    whhcn_h = nc.dram_tensor("whhcn", (H, 2 * G3), F32, kind="ExternalInput")
    wout_h = nc.dram_tensor("wout", (128, V), BF16, kind="ExternalInput")
    ones_h = nc.dram_tensor("onesrow", (1, 128), BF16, kind="ExternalInput")
    out_h = nc.dram_tensor("out", (T, V), BF16, kind="ExternalOutput")

    with tile.TileContext(nc) as tc:
        with ExitStack() as ctx:
            const = ctx.enter_context(tc.tile_pool(name="const", bufs=1))
            hall = ctx.enter_context(tc.tile_pool(name="hall", bufs=1))

            ident = const.tile([128, 128], F32, tag="ident")
            make_identity(nc, ident[:])
            wih_sb = const.tile([E + 1, 2 * G3], F32, tag="wih")
            nc.sync.dma_start(out=wih_sb[:], in_=wih_h[:])
            whhdd_sb = const.tile([H + 1, 2 * G3], F32, tag="whhdd")
            nc.sync.dma_start(out=whhdd_sb[:], in_=whhdd_h[:])
            whhcn_sb = const.tile([H, 2 * G3], F32, tag="whhcn")
            nc.sync.dma_start(out=whhcn_sb[:], in_=whhcn_h[:])
            tok_sb = const.tile([128, 4], I32, tag="tok")
            nc.sync.dma_start(out=tok_sb[:], in_=tok_h[:].rearrange("(g p) -> p g", p=128))

            woutc = hall.tile([128, V], BF16, tag="woutc")

            xt = const.tile([E + 1, T], F32, tag="xt")
            nc.vector.memset(xt[E:E + 1, :], 1.0)
            xtr = const.tile([E + 1, T], F32, tag="xtr")

            # Compact input-gate tensors, both direction blocks side by side.
            # Gate order is [z, r, n] so that tz lands at base partition 0
            # (BIR requires equal base partitions for two-SBUF-input ops).
            # gxc rows 0:64 = z/r gx (pad cols: z=+30, r=0); rows 64:96 = 0.
            # xnc = 2 * n-gate gx (pad cols 0).
            gxc = const.tile([G3, 2 * GXW], F32, tag="gxc")
            xnc = const.tile([H, 2 * GXW], F32, tag="xnc")
            for d in range(2):
                o = d * GXW
                nc.vector.memset(gxc[0:H, o:o + 2 * W], 30.0)
                nc.vector.memset(gxc[H:2 * H, o:o + 2 * W], 0.0)
                nc.vector.memset(xnc[:, o:o + 2 * W], 0.0)
            nc.vector.memset(gxc[2 * H:G3, :], 0.0)

            # h state by position, 32 partitions: cols [0, HBW) = L block,
            # [HBW, 2*HBW) = R block; real cols [2W, 2W+2S) within each block.
            hbufp = const.tile([H, 2 * HBW], F32, tag="hbufp")
            nc.vector.memset(hbufp[:], 0.0)

            # rings for zh' ([h;ones]), cn, cz, rz, n
            zhr, cnr, czr, rzr, nnr = [], [], [], [], []
            for j in range(3):
                zt = const.tile([H + 1, LAN], F32, tag=f"zh{j}", name=f"zh{j}")
                nc.vector.memset(zt[0:H, :], 0.0)
                nc.vector.memset(zt[H:H + 1, :], 1.0)
                zhr.append(zt)
                ct = const.tile([H, LAN], F32, tag=f"cn{j}", name=f"cn{j}")
                nc.vector.memset(ct[:], 0.0)
                cnr.append(ct)
                czt = const.tile([H, LAN], F32, tag=f"cz{j}", name=f"cz{j}")
                czr.append(czt)
                rzt = const.tile([2 * H, LAN], F32, tag=f"rz{j}", name=f"rz{j}")
                rzr.append(rzt)
                nnt = const.tile([H, LAN], F32, tag=f"nn{j}", name=f"nn{j}")
                nnr.append(nnt)
            uur = []
            for j in range(3):
                uut = const.tile([H, LAN], F32, tag=f"uu{j}", name=f"uu{j}")
                uur.append(uut)

            with (
                tc.tile_pool(name="gath", bufs=2) as gpool,
                tc.tile_pool(name="pps", bufs=2, space="PSUM") as ppre,
            ):
                if "pre" in phases:
                    # embedding gather + transpose to [E, tokens]
                    for g in range(4):
                        xg = gpool.tile([128, E], F32, tag="xg")
                        nc.gpsimd.indirect_dma_start(
                            out=xg[:], out_offset=None, in_=emb_h[:],
                            in_offset=bass.IndirectOffsetOnAxis(ap=tok_sb[:, g:g + 1], axis=0),
                        )
                        xps = ppre.tile([E, 128], F32, tag="ps")
                        nc.tensor.transpose(xps[:], xg[:], ident[:])
                        nc.scalar.copy(out=xt[0:E, g * 128:(g + 1) * 128], in_=xps[:])

                    # time-reversed copy (pairwise: s reversed, b kept)
                    src = xt[:]
                    nc.vector.tensor_copy(
                        out=xtr[:],
                        in_=ap3(src, 2 * (S - 1), [[-2, S], [1, 2]]),
                    )

                    # input-gate matmuls -> compact tiles
                    for d, rhs in ((0, xt), (1, xtr)):
                        o = d * GXW
                        prz = ppre.tile([2 * H, T], F32, tag="prz")
                        nc.tensor.matmul(prz[:], wih_sb[:, d * G3:d * G3 + 2 * H], rhs[:],
                                         start=True, stop=True)
                        nc.vector.tensor_copy(out=gxc[0:2 * H, o + 2 * W:o + 2 * W + T], in_=prz[:])
                        pn = ppre.tile([H, T], F32, tag="pn")
                        nc.tensor.matmul(pn[:], wih_sb[:, d * G3 + 2 * H:(d + 1) * G3], rhs[:],
                                         start=True, stop=True)
                        nc.vector.tensor_copy(out=xnc[:, o + 2 * W:o + 2 * W + T], in_=pn[:])

                # wout cache DMA, issued from the gpsimd queue AFTER the embed
                # gathers (the DMA-engine pool is a serial FIFO; this keeps the
                # big load from delaying the scan start).
                for c0 in range(0, V, 4096):
                    cw = min(4096, V - c0)
                    nc.gpsimd.dma_start(out=woutc[:, c0:c0 + cw], in_=wout_h[:][:, c0:c0 + cw])

            # ---- chunked fused scan ----
            with tc.tile_pool(name="scp", bufs=3, space="PSUM") as scp:
                for t in range(STEPS if "scan" in phases else 0):
                    pt = scp.tile([128, LAN], F32, tag="sp")
                    gh = pt[0:G3, 0:LAN]
                    uu = uur[t % 3][:]
                    zhp = zhr[(t - 1) % 3]
                    cnp = cnr[(t - 1) % 3]
                    zhc = zhr[t % 3]
                    cnc = cnr[t % 3]
                    czc = czr[t % 3]
                    rzv = rzr[t % 3][:]
                    nnv = nnr[t % 3][:]

                    # gates(t) = gx(t) + 0.5*Whh @ zh'(t-1) + Whh @ cn(t-1)
                    gsrc = ap3(gxc[0:G3, :], 2 * t, [[GXW, 2], [2 * P, C], [1, 2]])
                    nc.tensor.matmul(gh, ident[0:G3, 0:G3], gsrc,
                                     start=True, stop=False, skip_group_check=True)
                    for d in range(2):
                        nc.tensor.matmul(
                            gh[:, d * LPD:(d + 1) * LPD],
                            whhdd_sb[:, d * G3:(d + 1) * G3], zhp[:, d * LPD:(d + 1) * LPD],
                            start=False, stop=False, skip_group_check=True)
                    for d in range(2):
                        nc.tensor.matmul(
                            gh[:, d * LPD:(d + 1) * LPD],
                            whhcn_sb[:, d * G3:(d + 1) * G3], cnp[:, d * LPD:(d + 1) * LPD],
                            start=False, stop=(d == 1), skip_group_check=True)

                    # z,r = sigmoid = .5 + .5*tanh(x/2)  (gate order [z, r, n])
                    nc.scalar.activation(out=rzv, in_=pt[0:2 * H, 0:LAN], func=AF.Tanh, scale=0.5)

                    # n path first in the DVE stream (critical):
                    # u = (tr+1)*hn + 2*xn ; n = tanh(u/2)
                    nc.vector.scalar_tensor_tensor(
                        out=uu, in0=rzv[H:2 * H, :], scalar=1.0,
                        in1=pt[2 * H:G3, 0:LAN], op0=ALU.add, op1=ALU.mult)
                    usrc = ap3(xnc[0:H, :], 2 * t, [[GXW, 2], [2 * P, C], [1, 2]])
                    nc.vector.tensor_tensor(uu, uu, usrc, ALU.add)
                    nc.scalar.activation(out=nnv, in_=uu, func=AF.Tanh, scale=0.5)

                    # off-path: zh'(t) = (tz+1)*h(t-1) (skip at t=0: ring
                    # holds zeros), cz = (1-z) = .5 - .5*tz (Pool)
                    if t > 0:
                        hl = ap3(hbufp[0:H, :], 2 * (t - 1), [[2 * P, C], [1, 2]])
                        nc.vector.scalar_tensor_tensor(
                            out=zhc[0:H, 0:LPD], in0=rzv[0:H, 0:LPD],
                            scalar=1.0, in1=hl, op0=ALU.add, op1=ALU.mult)
                        hr = ap3(hbufp[0:H, :], HBW + (4 * W + 2 * S - 2) - 2 * (t - 1),
                                 [[-2 * P, C], [1, 2]])
                        nc.vector.scalar_tensor_tensor(
                            out=zhc[0:H, LPD:LAN], in0=rzv[0:H, LPD:LAN],
                            scalar=1.0, in1=hr, op0=ALU.add, op1=ALU.mult)
                    nc.gpsimd.tensor_scalar(czc[:], rzv[0:H, :], -0.5, 0.5,
                                            ALU.mult, ALU.add)

                    # cn = (1-z)*n ; h(t) = cn + 0.5*zh'
                    nc.vector.tensor_tensor(cnc[:], nnv, czc[:], ALU.mult)
                    hl = ap3(hbufp[0:H, :], 2 * t, [[2 * P, C], [1, 2]])
                    nc.vector.scalar_tensor_tensor(
                        out=hl, in0=zhc[0:H, 0:LPD], scalar=0.5, in1=cnc[:, 0:LPD],
                        op0=ALU.mult, op1=ALU.add)
                    hr = ap3(hbufp[0:H, :], HBW + (4 * W + 2 * S - 2) - 2 * t,
                             [[-2 * P, C], [1, 2]])
                    nc.vector.scalar_tensor_tensor(
                        out=hr, in0=zhc[0:H, LPD:LAN], scalar=0.5, in1=cnc[:, LPD:LAN],
                        op0=ALU.mult, op1=ALU.add)

            # ---- pack shells (bf16, 128 tokens):
            # rows [L-hi 0:32 | R-hi 32:64 | L-lo 64:96 | R-lo 96:127 | ones 127]
            hsh = []
            for k in range(4):
                hs = hall.tile([128, 128], BF16, tag=f"hs{k}", name=f"hs{k}")
                cl = 2 * W + 128 * k
                cr = HBW + 2 * W + 128 * k
                nc.vector.tensor_copy(out=hs[0:H, :], in_=hbufp[:, cl:cl + 128])
                nc.vector.tensor_copy(out=hs[H:2 * H, :], in_=hbufp[:, cr:cr + 128])
                nc.vector.tensor_tensor(hs[2 * H:G3, :], hbufp[:, cl:cl + 128],
                                        hs[0:H, :], ALU.subtract)
                hr16 = hall.tile([H, 128], BF16, tag=f"hr{k}", name=f"hr16_{k}")
                nc.vector.tensor_copy(out=hr16[:], in_=hbufp[:, cr:cr + 128])
                nc.vector.tensor_tensor(hr16[0:H - 1, :], hbufp[0:H - 1, cr:cr + 128],
                                        hr16[0:H - 1, :], ALU.subtract)
                nc.vector.tensor_copy(out=hs[G3:127, :], in_=hr16[0:H - 1, :])
                nc.sync.dma_start(out=hs[127:128, :], in_=ones_h[:])
                hsh.append(hs)

            do_proj = "proj" in phases
            if not do_proj and "scan" not in phases:
                for k in range(4):
                    nc.vector.memset(hsh[k][:], 0.0)

            # ---- projection + log-softmax ----
            opool = ctx.enter_context(tc.tile_pool(name="outp", bufs=4))
            with (
                tc.tile_pool(name="pp", bufs=NPSB, space="PSUM") as ppool,
                tc.tile_pool(name="esc", bufs=6) as epool,
            ):
                stats = [const.tile([128, NGRP], F32, tag=f"st{k}", name=f"stats{k}")
                         for k in range(4)]
                negc = [const.tile([128, 1], F32, tag=f"ng{k}", name=f"negc{k}")
                        for k in range(4)]

                def groups():
                    for g in range(NGRP):
                        c0 = g * VGRP
                        yield g, c0, min(VGRP, V - c0)

                def mms(k, g, c0, gw, tag):
                    ps = ppool.tile([128, VGRP], F32, tag="pp", name=f"pp{tag}{k}_{g}")
                    for q0 in range(0, gw, 512):
                        qw = min(512, gw - q0)
                        nc.tensor.matmul(ps[:, q0:q0 + qw], hsh[k][:],
                                         woutc[:, c0 + q0:c0 + q0 + qw],
                                         start=True, stop=True)
                    return ps

                def p1(k, g, c0, gw, dve_red=False):
                    ps = mms(k, g, c0, gw, "a")
                    esc = epool.tile([128, VGRP], BF16, tag="esc", name=f"esc{k}_{g}")
                    if dve_red:
                        # sum on DVE (2x mode on the bf16 scratch); saves the
                        # ACT accumulator-read overhead on the critical engine
                        nc.scalar.activation(out=esc[:, 0:gw], in_=ps[:, 0:gw], func=AF.Exp)
                        nc.vector.tensor_reduce(out=stats[k][:, g:g + 1], in_=esc[:, 0:gw],
                                                axis=mybir.AxisListType.X, op=ALU.add)
                    else:
                        nc.scalar.activation(out=esc[:, 0:gw], in_=ps[:, 0:gw], func=AF.Exp,
                                             accum_out=stats[k][:, g:g + 1])

                def lse(k):
                    # negc = -ln(ssum) without the Ln table (exp stays loaded):
                    # crude log from the float bits, then one Newton step
                    # y1 = y0 - 1 + s*exp(-y0); |err| <= ~5e-4.
                    ssum = const.tile([128, 1], F32, tag=f"ss{k}", name=f"ssum{k}")
                    nc.vector.tensor_reduce(out=ssum[:], in_=stats[k][:],
                                            axis=mybir.AxisListType.X, op=ALU.add)
                    y0 = const.tile([128, 1], F32, tag=f"y0{k}", name=f"y0_{k}")
                    nc.vector.tensor_copy(out=y0[:], in_=ssum[:].bitcast(I32))
                    nc.vector.tensor_scalar(y0[:], y0[:], 8.2629582e-8, -87.999887,
                                            ALU.mult, ALU.add)
                    ex = const.tile([128, 1], F32, tag=f"ex{k}", name=f"ex_{k}")
                    nc.scalar.activation(out=ex[:], in_=y0[:], func=AF.Exp, scale=-1.0)
                    nc.vector.tensor_tensor(ex[:], ex[:], ssum[:], ALU.mult)
                    nc.vector.tensor_tensor(ex[:], ex[:], y0[:], ALU.add)
                    nc.vector.tensor_scalar(negc[k][:], ex[:], -1.0, 1.0,
                                            ALU.mult, ALU.add)

                def p2(k, g, c0, gw, split=None):
                    ps = mms(k, g, c0, gw, "b")
                    ob = opool.tile([128, VGRP], BF16, tag="ob", name=f"ob{k}_{g}")
                    if split == "half":
                        # drain mode: ACT and DVE each finalize half the tile
                        h0 = (gw + 1) // 2
                        nc.scalar.activation(out=ob[:, 0:h0], in_=ps[:, 0:h0],
                                             func=AF.Identity, bias=negc[k][:, 0:1])
                        nc.vector.tensor_scalar_add(ob[:, h0:gw], ps[:, h0:gw],
                                                    negc[k][:, 0:1])
                    else:
                        split = fin_split if split is None else split
                        eng = split[g % len(split)]
                        if eng == "v":
                            nc.vector.tensor_scalar_add(ob[:, 0:gw], ps[:, 0:gw],
                                                        negc[k][:, 0:1])
                        else:
                            nc.scalar.activation(out=ob[:, 0:gw], in_=ps[:, 0:gw],
                                                 func=AF.Identity, bias=negc[k][:, 0:1])
                    dst = bass.AP(tensor=out_h[:].tensor, offset=(128 * k) * V + c0,
                                  ap=[[V, 128], [1, gw]])
                    nc.sync.dma_start(out=dst, in_=ob[:, 0:gw])

                if do_proj:
                    for g, c0, gw in groups():
                        p1(0, g, c0, gw, dve_red=True)
                    for k in range(4):
                        lse(k)
                        if k < 3:
                            for g, c0, gw in groups():
                                p1(k + 1, g, c0, gw)
                                p2(k, g, c0, gw)
                        else:
                            # drain: no pass-1 left; ACT and DVE each take
                            # half of every finalize tile
                            for g, c0, gw in groups():
                                p2(k, g, c0, gw, split="half")
    nc.compile()
    return nc


_CACHE = {}


def _get_module():
    if "nc" not in _CACHE:
        _CACHE["nc"] = build_module()
    return _CACHE["nc"]


def prep_inputs(inputs):
    """Host-side prep: build per-core input maps from the full input dict."""
    import ml_dtypes

    ib = np.asarray(inputs["input_batch"])
    embed = np.ascontiguousarray(np.asarray(inputs["embed"], dtype=np.float32))

    def f32(x):
        return np.asarray(x, dtype=np.float32)

    # Gate order [z, r, n] (see kernel); n-gate input weights pre-doubled.
    perm = np.concatenate([np.arange(H, 2 * H), np.arange(0, H),
                           np.arange(2 * H, G3)])
    nsc = np.concatenate([np.ones(2 * H, np.float32), 2.0 * np.ones(H, np.float32)])

    wih = np.zeros((E + 1, 2 * G3), np.float32)
    for d, (Wd, bd) in enumerate(((inputs["Wl_ih"], inputs["bl_ih"]),
                                  (inputs["Wr_ih"], inputs["br_ih"]))):
        Wd, bd = f32(Wd), f32(bd)
        o = d * G3
        wih[:E, o:o + G3] = Wd[:, perm] * nsc
        wih[E, o:o + G3] = bd[perm] * nsc

    whhdd = np.zeros((H + 1, 2 * G3), np.float32)
    whhcn = np.zeros((H, 2 * G3), np.float32)
    for d, (Wd, bd) in enumerate(((inputs["Wl_hh"], inputs["bl_hh"]),
                                  (inputs["Wr_hh"], inputs["br_hh"]))):
        Wd, bd = f32(Wd), f32(bd)
        o = d * G3
        whhdd[:H, o:o + G3] = 0.5 * Wd[:, perm]
        whhdd[H, o:o + G3] = bd[perm]
        whhcn[:, o:o + G3] = Wd[:, perm]

    rnn_out = f32(inputs["rnn_out"])
    rnn_out_bias = f32(inputs["rnn_out_bias"])
    wout = np.zeros((128, V), np.float32)
    wout[0:2 * H] = rnn_out                  # hi rows (L then R)
    wout[2 * H:127] = rnn_out[0:2 * H - 1]   # lo rows (L 32, R 31)
    wout[127] = rnn_out_bias[0]              # ones row
    woutp = wout.astype(ml_dtypes.bfloat16)

    in_maps = []
    for c in range(NCORES):
        tok = np.ascontiguousarray(
            ib[:, BC * c:BC * (c + 1)].astype(np.int32).reshape(T)
        )
        in_maps.append(
            {"tok": tok, "embed": embed, "wih": wih, "whhdd": whhdd,
             "whhcn": whhcn, "wout": woutp,
             "onesrow": np.ones((1, 128), ml_dtypes.bfloat16)}
        )
    return in_maps


def assemble_output(results):
    out = np.empty((S, B, V), np.float32)
    for c in range(NCORES):
        out[:, BC * c:BC * (c + 1), :] = (
            results[c]["out"].astype(np.float32).reshape(S, BC, V)
        )
    return out


def kernel(**inputs):
    from concourse.bass_utils import run_bass_kernel_spmd

    nc = _get_module()
    in_maps = prep_inputs(inputs)
    res = run_bass_kernel_spmd(nc, in_maps, core_ids=list(range(NCORES)))
    return assemble_output(res.results)
